# revision 11
# baseline (speedup 1.0000x reference)
"""Trainium2 Bass kernel for DkNetCL (4x [3x3 conv + SRePro] + FC 32768->10).

Strategy v2 (pure data parallel over 8 cores, 128 images/core):
- Activations live COMPACT on SBUF: [128 part = 4img x 32ch, 32x32 pix in
  free dim]. Convs are block-diagonal matmuls (lhsT [128,128] per 3x3 tap);
  border taps are TRIMMED (smaller output windows) instead of using padded
  buffers, so no guard zones, no memsets, no padded copies. Tap (0,0) runs
  first with start=True covering the full PSUM range; the other 8 accumulate
  into sub-windows.
- Layer 0 (3->32 ch) reads x directly: x is DMA'd once per sub-batch into
  [128, 1024] blocks (partition 32q+3il+c for group q, image il, channel c),
  and L0 is 9 trimmed tap-matmuls with K=32-blocks (12 used rows).
- SRePro is DEFERRED: per-image scale factors commute through convs, so the
  conv chain runs unscaled (plain PSUM->SBUF copies); ACT computes per-layer
  squared norms (accum) on the fly; after L3 a single ones-block-diag matmul
  + short DVE chain reconstructs the product of the 4 scale factors, applied
  once when staging the FC input.
- FC path in bf16: PE transposes + 256 accumulating matmuls (1 cyc/row vs 4
  for fp32r at N=10).
"""

import numpy as np

CORES = 8
B_PER_CORE = 128
SUB = 32            # images per sub-batch
NSUB = B_PER_CORE // SUB
NGRP = SUB // 4     # groups of 4 images per sub-batch
TAPS = [(dy, dx) for dy in (-1, 0, 1) for dx in (-1, 0, 1)]
TAPS_ORD = [(0, 0)] + [t for t in TAPS if t != (0, 0)]

MM_DT = "float32r"  # dtype used for conv matmul operands
REPEAT = 0          # >0: wrap main loop in a hardware For_i for timing


def build_bass():
    import concourse.bass as bass
    import concourse.mybir as mybir
    import concourse.tile as tile
    from concourse import bacc

    f32 = mybir.dt.float32
    bf16 = mybir.dt.bfloat16
    mdt = getattr(mybir.dt, MM_DT)
    AF = mybir.ActivationFunctionType
    ALU = mybir.AluOpType

    nc = bacc.Bacc("TRN2", target_bir_lowering=False, debug=False)

    x_d = nc.dram_tensor("x", [B_PER_CORE, 3, 32, 32], f32, kind="ExternalInput")
    w0_d = nc.dram_tensor("conv_w0", [32, 3, 3, 3], f32, kind="ExternalInput")
    w_d = {i: nc.dram_tensor(f"conv_w{i}", [32, 32, 3, 3], f32,
                             kind="ExternalInput") for i in (1, 2, 3)}
    fcw_d = nc.dram_tensor("fc_w", [10, 32768], f32, kind="ExternalInput")
    fcb_d = nc.dram_tensor("fc_b", [10], f32, kind="ExternalInput")
    y_d = nc.dram_tensor("y", [B_PER_CORE, 10], f32, kind="ExternalOutput")

    def dmt(ap):
        return ap.bitcast(mdt) if MM_DT != "float32" else ap

    def trim(dy, dx):
        return (max(0, -dy), 32 - max(0, dy), max(0, -dx), 32 - max(0, dx))

    with tile.TileContext(nc) as tc:
        with (
            tc.tile_pool(name="persist", bufs=1) as pp,
            tc.tile_pool(name="work", bufs=3) as wp,
            tc.tile_pool(name="cpsum", bufs=3, space="PSUM") as cpool,
            tc.tile_pool(name="spsum", bufs=1, space="PSUM") as spool,
        ):
            # ---------- persistent tiles ----------
            # x: partition 32q + 3il + c (12 used rows per 32-block),
            # free (sub_half sh=2s+h, y, x)
            x_raw = pp.tile([128, 2 * NSUB * 32 * 34], mdt, tag="x_raw")
            actA = pp.tile([128, NGRP * 32 * 34], mdt, tag="actA")
            actB = pp.tile([128, NGRP * 32 * 34], mdt, tag="actB")
            aTv_t = pp.tile([128, 8 * 32 * SUB], bf16, tag="actT")
            fcst = pp.tile([128, 32 * 8 * 10], f32, tag="fcst")
            fc_sb = pp.tile([128, 32 * 8 * 10], bf16, tag="fc_sb")
            bias_sb = pp.tile([SUB, 10], f32, tag="bias")
            w0blk = pp.tile([128, 9 * 128], mdt, tag="w0blk")
            wbd = {(L, t): pp.tile([128, 128], mdt, tag=f"w{L}_{t}",
                                   name=f"w{L}_{t}")
                   for L in (1, 2, 3) for t in range(9)}
            ones_bd = pp.tile([128, 128], f32, tag="ones")
            sumsel = pp.tile([128, 32], bf16, tag="sumsel")
            ident = pp.tile([128, 128], f32, tag="ident")
            identb = pp.tile([128, 128], bf16, tag="identb")
            iota_a = pp.tile([128, 128], mybir.dt.int32, tag="iota_a")
            iota_b = pp.tile([128, 128], mybir.dt.int32, tag="iota_b")

            # ---------- init: weights, identity, ones ----------
            nc.vector.memset(w0blk[:].bitcast(f32), 0.0)
            nc.vector.memset(x_raw[:].bitcast(f32), 0.0)
            nc.vector.memset(actA[:].bitcast(f32), 0.0)
            nc.vector.memset(actB[:].bitcast(f32), 0.0)
            nc.vector.memset(ones_bd[:], 0.0)
            for j in range(4):
                nc.vector.memset(ones_bd[32*j:32*j+32, 32*j:32*j+32], 1.0)

            nc.gpsimd.iota(iota_a[:], pattern=[[1, 128]], base=0,
                           channel_multiplier=0)
            nc.gpsimd.iota(iota_b[:], pattern=[[0, 128]], base=0,
                           channel_multiplier=1)
            nc.vector.tensor_tensor(ident[:], iota_a[:], iota_b[:],
                                    ALU.is_equal)
            nc.vector.tensor_copy(identb[:], ident[:])
            for j in range(4):
                nc.vector.tensor_copy(sumsel[32*j:32*j+32, :],
                                      ident[0:32, 0:32])

            # w0blk[32q + 3il + c, 128t + 32il + co] = w0[co, c, t], 4 q-copies
            w0src = w0_d[:].rearrange("o c dy dx -> (dy dx) c o")
            for q in range(4):
                for t in range(9):
                    for il in range(4):
                        nc.sync.dma_start(
                            out=w0blk[32*q+3*il:32*q+3*il+3,
                                      128*t+32*il:128*t+32*il+32],
                            in_=dmt(w0src[t]))
            # conv_w{1..3} -> wbd[(L,t)][32j+ci, 32j+co] = w[co, ci, dy, dx]
            for L in (1, 2, 3):
                wsrc = w_d[L][:].rearrange("co ci dy dx -> (dy dx) ci co")
                for t in range(9):
                    nc.vector.memset(wbd[(L, t)][:].bitcast(f32), 0.0)
                    for j in range(4):
                        nc.sync.dma_start(
                            out=wbd[(L, t)][32*j:32*j+32, 32*j:32*j+32],
                            in_=dmt(wsrc[t]))

            # fc_w -> fcst[p, (ch, c8, o)] = fc_w[o, ch*1024 + c8*128 + p]
            fstv = fcst[:].rearrange("p (ch c8 o) -> p ch c8 o", ch=32, c8=8)
            fsrc = fcw_d[:].rearrange("o (ch c8 p) -> ch c8 p o", ch=32, c8=8)
            for ch in range(32):
                for c8 in range(8):
                    nc.sync.dma_start(out=fstv[:, ch, c8], in_=fsrc[ch, c8])
            nc.vector.tensor_copy(fc_sb[:], fcst[:])  # f32 -> bf16
            fcv = fc_sb[:].rearrange("p (ch c8 o) -> p ch c8 o", ch=32, c8=8)
            for i in range(SUB):
                nc.sync.dma_start(out=bias_sb[i:i+1, :], in_=fcb_d[None, :])

            # ---------- matmul-dtype views ----------
            xw = x_raw[:].rearrange("p (sh y x) -> p sh y x", sh=8, y=32)
            xdv = x_raw[:].rearrange("(q rr) (sh y x) -> q rr sh y x",
                                     q=4, sh=8, y=32)
            avAm = actA[:].rearrange("p (g y x) -> p g y x", g=NGRP, y=32)
            avBm = actB[:].rearrange("p (g y x) -> p g y x", g=NGRP, y=32)
            avBf = actB[:].bitcast(f32).rearrange("p (g y x) -> p g y x",
                                                  g=NGRP, y=32)
            w0m = w0blk[:]
            wbdm = {k: v[:] for k, v in wbd.items()}
            aTv = aTv_t[:].rearrange("p (c8 ch i) -> p c8 ch i", c8=8, ch=32)

            import contextlib
            rep_ctx = tc.For_i(0, REPEAT, 1) if REPEAT else \
                contextlib.nullcontext()
            with rep_ctx:
              for s in range(NSUB):
                # ---- load x for this sub-batch: 2 DMAs (halves) ----
                for h in range(2):
                    src = x_d[32*s+16*h: 32*s+16*h+16].rearrange(
                        "(q il) c y x -> q (il c) y x", q=4)
                    for qq in range(4):
                        nc.sync.dma_start(
                            out=xdv[qq, 0:12, 2*s+h, :, 1:33],
                            in_=dmt(src[qq]))

                nbufs = wp.tile([128, 4 * NGRP], f32, tag="nbufs")
                nbv = nbufs[:].rearrange("p (g l) -> p g l", g=NGRP)

                # ---- conv layers ----
                for L in range(4):
                    srcm = avBm if L % 2 == 0 else avAm
                    dstv = avAm if L % 2 == 0 else avBm
                    # pairs share each stationary load (tap-outer order);
                    # L0 pairs (p, p+4) keep the same q so lhsT is reused too
                    pairs = [(p, p + 4) for p in range(4)] if L == 0 else                             [(2*p, 2*p + 1) for p in range(4)]
                    for pair in pairs:
                        pss = {}
                        for g in pair:
                            pss[g] = cpool.tile([128, 1024], f32, tag="conv", name=f"ps{g}")
                        for ti, (dy, dx) in enumerate(TAPS_ORD):
                            t = TAPS.index((dy, dx))
                            for g in pair:
                                q, h = g % 4, g // 4
                                psv = pss[g][:].rearrange(
                                    "p (y x) -> p y x", y=32)
                                for cy in range(2):
                                    y0 = max(max(0, -dy), 16*cy)
                                    y1 = min(32 - max(0, dy), 16*cy+16)
                                    if L == 0:
                                        lhsT = w0m[32*q:32*q+32,
                                                   128*t:128*t+128]
                                        rhs = xw[32*q:32*q+32, 2*s+h,
                                                 y0+dy:y1+dy, 1+dx:33+dx]
                                        tp = (32 * q, 0)
                                    else:
                                        lhsT = wbdm[(L, t)]
                                        rhs = srcm[:, g, y0+dy:y1+dy,
                                                   1+dx:33+dx]
                                        tp = None
                                    nc.tensor.matmul(
                                        psv[:, y0:y1, :], lhsT, rhs,
                                        start=(ti == 0), stop=(ti == 8),
                                        tile_position=tp,
                                        skip_group_check=True)
                        for g in pair:
                            ps = pss[g]
                            psv = ps[:].rearrange("p (y x) -> p y x", y=32)
                            # squared norm of the unscaled conv output (ACT)
                            scr = wp.tile([128, 1024], f32, tag="scr")
                            nc.scalar.activation(scr[:], ps[:], AF.Square,
                                                 accum_out=nbv[:, g, L:L+1])
                            # plain copy PSUM -> SBUF, alternating DVE/ACT
                            if L % 2 == 0:
                                nc.vector.tensor_copy(dstv[:, g, :, 1:33],
                                                      psv[:])
                            else:
                                nc.scalar.activation(dstv[:, g, :, 1:33],
                                                     psv[:], AF.Copy)

                # ---- deferred SRePro: P4 per image, batched over groups ----
                nrm_ps = spool.tile([128, 4 * NGRP], f32, tag="small", name="nrm_ps")
                nc.tensor.matmul(nrm_ps[:], ones_bd[:], nbufs[:],
                                 start=True, stop=True)
                nrm = wp.tile([128, 4 * NGRP], f32, tag="nrm_sb")
                nc.vector.tensor_copy(nrm[:], nrm_ps[:])
                nv = nrm[:].rearrange("p (g l) -> p g l", g=NGRP)
                acc = wp.tile([128, NGRP], f32, tag="acc")
                tmp = wp.tile([128, NGRP], f32, tag="tmp")
                # P1 = 1/(1 + N1/2)
                nc.vector.tensor_scalar(acc[:], nv[:, :, 0], 0.5, 1.0,
                                        ALU.mult, ALU.add)
                nc.vector.reciprocal(acc[:], acc[:])
                for j in (1, 2, 3):
                    nc.vector.tensor_tensor(tmp[:], acc[:], acc[:], ALU.mult)
                    nc.vector.tensor_tensor(tmp[:], tmp[:], nv[:, :, j],
                                            ALU.mult)
                    nc.vector.tensor_scalar(tmp[:], tmp[:], 0.5, 1.0,
                                            ALU.mult, ALU.add)
                    nc.vector.reciprocal(tmp[:], tmp[:])
                    nc.vector.tensor_tensor(acc[:], acc[:], tmp[:], ALU.mult)

                # ---- FC staging: scale, transpose to [pix, img] ----
                for g in range(NGRP):
                    comp = wp.tile([128, 1024], bf16, tag="comp")
                    compv = comp[:].rearrange("p (y x) -> p y x", y=32)
                    nc.gpsimd.tensor_scalar(
                        compv[:], avBf[:, g, :, 1:33],
                        acc[:, g:g+1], None, ALU.mult)
                    for c8 in range(8):
                        pT = spool.tile([128, 128], bf16, tag="small", name="pT")
                        nc.tensor.transpose(pT[:], comp[:, 128*c8:128*(c8+1)],
                                            identb[:])
                        nc.vector.tensor_copy(
                            aTv[:, c8, :, 4*g:4*g+4],
                            pT[:].rearrange("p (il ch) -> p ch il", il=4))

                # ---- FC: 4 concurrent col-tiled chains + combine ----
                fcp = spool.tile([128, 10], f32, tag="small", name="fcp")
                for k in range(256):
                    j = k % 4
                    pair = 64 * j + k // 4
                    c8, ch = pair // 32, pair % 32
                    nc.tensor.matmul(
                        fcp[32*j:32*j+32, :], aTv[:, c8, ch, :],
                        fcv[:, ch, c8, :],
                        start=(k < 4), stop=(k >= 252),
                        tile_position=(0, 32 * j), skip_group_check=True)
                fcp_sb = wp.tile([128, 10], bf16, tag="fcp_sb")
                nc.vector.tensor_copy(fcp_sb[:], fcp[:])
                yp = spool.tile([SUB, 10], f32, tag="small", name="yp")
                nc.tensor.matmul(yp[:], sumsel[:], fcp_sb[:],
                                 start=True, stop=True)
                y_sb = wp.tile([SUB, 10], f32, tag="ysb")
                nc.vector.tensor_tensor(y_sb[:], yp[:], bias_sb[:], ALU.add)
                nc.sync.dma_start(out=y_d[SUB*s:SUB*(s+1), :], in_=y_sb[:])

    nc.compile()
    return nc


_NC_CACHE = None


def kernel(**inputs):
    global _NC_CACHE
    from concourse.bass_utils import run_bass_kernel_spmd

    if _NC_CACHE is None:
        _NC_CACHE = build_bass()
    nc = _NC_CACHE

    x = np.ascontiguousarray(inputs["x"], np.float32)
    shared = {k: np.ascontiguousarray(np.asarray(inputs[k]), np.float32)
              for k in ("conv_w0", "conv_w1", "conv_w2", "conv_w3",
                        "fc_w", "fc_b")}
    in_maps = [
        {"x": x[i*B_PER_CORE:(i+1)*B_PER_CORE], **shared} for i in range(CORES)
    ]
    res = run_bass_kernel_spmd(nc, in_maps, core_ids=list(range(CORES)))
    return np.concatenate([r["y"] for r in res.results], axis=0)


# revision 12
# speedup vs baseline: 1.2163x; 1.2163x over previous
"""Trainium2 Bass kernel for DkNetCL (4x [3x3 conv + SRePro] + FC 32768->10).

Strategy v2 (pure data parallel over 8 cores, 128 images/core):
- Activations live COMPACT on SBUF: [128 part = 4img x 32ch, 32x32 pix in
  free dim]. Convs are block-diagonal matmuls (lhsT [128,128] per 3x3 tap);
  border taps are TRIMMED (smaller output windows) instead of using padded
  buffers, so no guard zones, no memsets, no padded copies. Tap (0,0) runs
  first with start=True covering the full PSUM range; the other 8 accumulate
  into sub-windows.
- Layer 0 (3->32 ch) reads x directly: x is DMA'd once per sub-batch into
  [128, 1024] blocks (partition 32q+3il+c for group q, image il, channel c),
  and L0 is 9 trimmed tap-matmuls with K=32-blocks (12 used rows).
- SRePro is DEFERRED: per-image scale factors commute through convs, so the
  conv chain runs unscaled (plain PSUM->SBUF copies); ACT computes per-layer
  squared norms (accum) on the fly; after L3 a single ones-block-diag matmul
  + short DVE chain reconstructs the product of the 4 scale factors, applied
  once when staging the FC input.
- FC path in bf16: PE transposes + 256 accumulating matmuls (1 cyc/row vs 4
  for fp32r at N=10).
"""

import numpy as np

CORES = 8
B_PER_CORE = 128
SUB = 32            # images per sub-batch
NSUB = B_PER_CORE // SUB
NGRP = SUB // 4     # groups of 4 images per sub-batch
TAPS = [(dy, dx) for dy in (-1, 0, 1) for dx in (-1, 0, 1)]
TAPS_ORD = [(0, 0)] + [t for t in TAPS if t != (0, 0)]

MM_DT = "float32r"  # dtype used for conv matmul operands
REPEAT = 0          # >0: wrap main loop in a hardware For_i for timing
ABLATE = ""         # "tail": skip transposes+FC; "conv": skip convs


def build_bass():
    import concourse.bass as bass
    import concourse.mybir as mybir
    import concourse.tile as tile
    from concourse import bacc

    f32 = mybir.dt.float32
    bf16 = mybir.dt.bfloat16
    mdt = getattr(mybir.dt, MM_DT)
    AF = mybir.ActivationFunctionType
    ALU = mybir.AluOpType

    nc = bacc.Bacc("TRN2", target_bir_lowering=False, debug=False)

    x_d = nc.dram_tensor("x", [B_PER_CORE, 3, 32, 32], f32, kind="ExternalInput")
    w0_d = nc.dram_tensor("conv_w0", [32, 3, 3, 3], f32, kind="ExternalInput")
    w_d = {i: nc.dram_tensor(f"conv_w{i}", [32, 32, 3, 3], f32,
                             kind="ExternalInput") for i in (1, 2, 3)}
    fcw_d = nc.dram_tensor("fc_w", [10, 32768], f32, kind="ExternalInput")
    fcb_d = nc.dram_tensor("fc_b", [10], f32, kind="ExternalInput")
    y_d = nc.dram_tensor("y", [B_PER_CORE, 10], f32, kind="ExternalOutput")

    def dmt(ap):
        return ap.bitcast(mdt) if MM_DT != "float32" else ap

    def trim(dy, dx):
        return (max(0, -dy), 32 - max(0, dy), max(0, -dx), 32 - max(0, dx))

    with tile.TileContext(nc) as tc:
        with (
            tc.tile_pool(name="persist", bufs=1) as pp,
            tc.tile_pool(name="work", bufs=3) as wp,
            tc.tile_pool(name="cpsum", bufs=3, space="PSUM") as cpool,
            tc.tile_pool(name="spsum", bufs=1, space="PSUM") as spool,
        ):
            # ---------- persistent tiles ----------
            # x: partition 32q + 3il + c (12 used rows per 32-block),
            # free (sub_half sh=2s+h, y, x)
            x_raw = pp.tile([128, 2 * NSUB * 32 * 34], mdt, tag="x_raw")
            actA = pp.tile([128, NGRP * 32 * 34], mdt, tag="actA")
            actB = pp.tile([128, NGRP * 32 * 34], mdt, tag="actB")
            aTv_t = pp.tile([128, 8 * 32 * SUB], bf16, tag="actT")
            fcst = pp.tile([128, 32 * 8 * 10], f32, tag="fcst")
            fc_sb = pp.tile([128, 32 * 8 * 10], bf16, tag="fc_sb")
            bias_sb = pp.tile([SUB, 10], f32, tag="bias")
            w0blk = pp.tile([128, 9 * 128], mdt, tag="w0blk")
            wbd = {(L, t): pp.tile([128, 128], mdt, tag=f"w{L}_{t}",
                                   name=f"w{L}_{t}")
                   for L in (1, 2, 3) for t in range(9)}
            ones_bd = pp.tile([128, 128], f32, tag="ones")
            sumsel = pp.tile([128, 32], bf16, tag="sumsel")
            ident = pp.tile([128, 128], f32, tag="ident")
            identb = pp.tile([128, 128], bf16, tag="identb")
            iota_a = pp.tile([128, 128], mybir.dt.int32, tag="iota_a")
            iota_b = pp.tile([128, 128], mybir.dt.int32, tag="iota_b")

            # ---------- init: weights, identity, ones ----------
            nc.vector.memset(w0blk[:].bitcast(f32), 0.0)
            nc.vector.memset(x_raw[:].bitcast(f32), 0.0)
            nc.vector.memset(actA[:].bitcast(f32), 0.0)
            nc.vector.memset(actB[:].bitcast(f32), 0.0)
            nc.vector.memset(ones_bd[:], 0.0)
            for j in range(4):
                nc.vector.memset(ones_bd[32*j:32*j+32, 32*j:32*j+32], 1.0)

            nc.gpsimd.iota(iota_a[:], pattern=[[1, 128]], base=0,
                           channel_multiplier=0)
            nc.gpsimd.iota(iota_b[:], pattern=[[0, 128]], base=0,
                           channel_multiplier=1)
            nc.vector.tensor_tensor(ident[:], iota_a[:], iota_b[:],
                                    ALU.is_equal)
            nc.vector.tensor_copy(identb[:], ident[:])
            for j in range(4):
                nc.vector.tensor_copy(sumsel[32*j:32*j+32, :],
                                      ident[0:32, 0:32])

            # w0blk[32q + 3il + c, 128t + 32il + co] = w0[co, c, t], 4 q-copies
            w0src = w0_d[:].rearrange("o c dy dx -> (dy dx) c o")
            for q in range(4):
                for t in range(9):
                    for il in range(4):
                        nc.sync.dma_start(
                            out=w0blk[32*q+3*il:32*q+3*il+3,
                                      128*t+32*il:128*t+32*il+32],
                            in_=dmt(w0src[t]))
            # conv_w{1..3} -> wbd[(L,t)][32j+ci, 32j+co] = w[co, ci, dy, dx]
            for L in (1, 2, 3):
                wsrc = w_d[L][:].rearrange("co ci dy dx -> (dy dx) ci co")
                for t in range(9):
                    nc.vector.memset(wbd[(L, t)][:].bitcast(f32), 0.0)
                    for j in range(4):
                        nc.sync.dma_start(
                            out=wbd[(L, t)][32*j:32*j+32, 32*j:32*j+32],
                            in_=dmt(wsrc[t]))

            # fc_w -> fcst[p, (ch, c8, o)] = fc_w[o, ch*1024 + c8*128 + p]
            fstv = fcst[:].rearrange("p (ch c8 o) -> p ch c8 o", ch=32, c8=8)
            fsrc = fcw_d[:].rearrange("o (ch c8 p) -> ch c8 p o", ch=32, c8=8)
            for ch in range(32):
                for c8 in range(8):
                    nc.sync.dma_start(out=fstv[:, ch, c8], in_=fsrc[ch, c8])
            nc.vector.tensor_copy(fc_sb[:], fcst[:])  # f32 -> bf16
            fcv = fc_sb[:].rearrange("p (ch c8 o) -> p ch c8 o", ch=32, c8=8)
            for i in range(SUB):
                nc.sync.dma_start(out=bias_sb[i:i+1, :], in_=fcb_d[None, :])

            # ---------- matmul-dtype views ----------
            xw = x_raw[:].rearrange("p (sh y x) -> p sh y x", sh=8, y=32)
            xdv = x_raw[:].rearrange("(q rr) (sh y x) -> q rr sh y x",
                                     q=4, sh=8, y=32)
            avAm = actA[:].rearrange("p (g y x) -> p g y x", g=NGRP, y=32)
            avBm = actB[:].rearrange("p (g y x) -> p g y x", g=NGRP, y=32)
            avBf = actB[:].bitcast(f32).rearrange("p (g y x) -> p g y x",
                                                  g=NGRP, y=32)
            w0m = w0blk[:]
            wbdm = {k: v[:] for k, v in wbd.items()}
            aTv = aTv_t[:].rearrange("p (c8 ch i) -> p c8 ch i", c8=8, ch=32)

            import contextlib
            rep_ctx = tc.For_i(0, REPEAT, 1) if REPEAT else \
                contextlib.nullcontext()
            with rep_ctx:
              for s in range(NSUB):
                # ---- load x for this sub-batch: 2 DMAs (halves) ----
                for h in range(2):
                    src = x_d[32*s+16*h: 32*s+16*h+16].rearrange(
                        "(q il) c y x -> q (il c) y x", q=4)
                    for qq in range(4):
                        nc.sync.dma_start(
                            out=xdv[qq, 0:12, 2*s+h, :, 1:33],
                            in_=dmt(src[qq]))

                nbufs = wp.tile([128, 4 * NGRP], f32, tag="nbufs")
                nbv = nbufs[:].rearrange("p (g l) -> p g l", g=NGRP)

                # ---- conv layers ----
                for L in range(4):
                    srcm = avBm if L % 2 == 0 else avAm
                    dstv = avAm if L % 2 == 0 else avBm
                    # pairs share each stationary load (tap-outer order);
                    # L0 pairs (p, p+4) keep the same q so lhsT is reused too
                    pairs = [(p, p + 4) for p in range(4)] if L == 0 else                             [(2*p, 2*p + 1) for p in range(4)]
                    for pair in pairs:
                        pss = {}
                        for g in pair:
                            pss[g] = cpool.tile([128, 1024], f32, tag="conv", name=f"ps{g}")
                        for ti, (dy, dx) in enumerate(TAPS_ORD):
                            t = TAPS.index((dy, dx))
                            for g in pair:
                                q, h = g % 4, g // 4
                                psv = pss[g][:].rearrange(
                                    "p (y x) -> p y x", y=32)
                                for cy in range(2):
                                    y0 = max(max(0, -dy), 16*cy)
                                    y1 = min(32 - max(0, dy), 16*cy+16)
                                    if L == 0:
                                        lhsT = w0m[32*q:32*q+32,
                                                   128*t:128*t+128]
                                        rhs = xw[32*q:32*q+32, 2*s+h,
                                                 y0+dy:y1+dy, 1+dx:33+dx]
                                        tp = (32 * q, 0)
                                    else:
                                        lhsT = wbdm[(L, t)]
                                        rhs = srcm[:, g, y0+dy:y1+dy,
                                                   1+dx:33+dx]
                                        tp = None
                                    nc.tensor.matmul(
                                        psv[:, y0:y1, :], lhsT, rhs,
                                        start=(ti == 0), stop=(ti == 8),
                                        tile_position=tp,
                                        skip_group_check=True)
                        for g in pair:
                            ps = pss[g]
                            psv = ps[:].rearrange("p (y x) -> p y x", y=32)
                            # squared norm of the unscaled conv output (ACT)
                            scr = wp.tile([128, 1024], f32, tag="scr")
                            nc.scalar.activation(scr[:], ps[:], AF.Square,
                                                 accum_out=nbv[:, g, L:L+1])
                            # plain copy PSUM -> SBUF, alternating DVE/ACT
                            if L % 2 == 0:
                                nc.vector.tensor_copy(dstv[:, g, :, 1:33],
                                                      psv[:])
                            else:
                                nc.scalar.activation(dstv[:, g, :, 1:33],
                                                     psv[:], AF.Copy)

                # ---- deferred SRePro: P4 per image, batched over groups ----
                nrm_ps = spool.tile([128, 4 * NGRP], f32, tag="small", name="nrm_ps")
                nc.tensor.matmul(nrm_ps[:], ones_bd[:], nbufs[:],
                                 start=True, stop=True)
                nrm = wp.tile([128, 4 * NGRP], f32, tag="nrm_sb")
                nc.vector.tensor_copy(nrm[:], nrm_ps[:])
                nv = nrm[:].rearrange("p (g l) -> p g l", g=NGRP)
                acc = wp.tile([128, NGRP], f32, tag="acc")
                tmp = wp.tile([128, NGRP], f32, tag="tmp")
                # P1 = 1/(1 + N1/2)
                nc.vector.tensor_scalar(acc[:], nv[:, :, 0], 0.5, 1.0,
                                        ALU.mult, ALU.add)
                nc.vector.reciprocal(acc[:], acc[:])
                for j in (1, 2, 3):
                    nc.vector.tensor_tensor(tmp[:], acc[:], acc[:], ALU.mult)
                    nc.vector.tensor_tensor(tmp[:], tmp[:], nv[:, :, j],
                                            ALU.mult)
                    nc.vector.tensor_scalar(tmp[:], tmp[:], 0.5, 1.0,
                                            ALU.mult, ALU.add)
                    nc.vector.reciprocal(tmp[:], tmp[:])
                    nc.vector.tensor_tensor(acc[:], acc[:], tmp[:], ALU.mult)

                # ---- FC staging: scale, transpose to [pix, img] ----
                for g in range(NGRP) if ABLATE != "tail" else []:
                    comp = wp.tile([128, 1024], bf16, tag="comp")
                    compv = comp[:].rearrange("p (y x) -> p y x", y=32)
                    nc.gpsimd.tensor_scalar(
                        compv[:], avBf[:, g, :, 1:33],
                        acc[:, g:g+1], None, ALU.mult)
                    for c8 in range(8):
                        pT = spool.tile([128, 128], bf16, tag="small", name="pT")
                        nc.tensor.transpose(pT[:], comp[:, 128*c8:128*(c8+1)],
                                            identb[:])
                        nc.vector.tensor_copy(
                            aTv[:, c8, :, 4*g:4*g+4],
                            pT[:].rearrange("p (il ch) -> p ch il", il=4))

                # ---- FC ----
                if ABLATE == "tail":
                    y_sb = wp.tile([SUB, 10], f32, tag="ysb")
                    nc.vector.tensor_scalar(y_sb[:], bias_sb[:],
                                            acc[:SUB, 0:1], None, ALU.add)
                    nc.sync.dma_start(out=y_d[SUB*s:SUB*(s+1), :], in_=y_sb[:])
                    continue
                yp = spool.tile([SUB, 10], f32, tag="small", name="yp")
                nmm = 0
                for c8 in range(8):
                    for ch in range(32):
                        nc.tensor.matmul(
                            yp[:], aTv[:, c8, ch, :], fcv[:, ch, c8, :],
                            start=(nmm == 0), stop=(nmm == 255))
                        nmm += 1
                y_sb = wp.tile([SUB, 10], f32, tag="ysb")
                nc.vector.tensor_tensor(y_sb[:], yp[:], bias_sb[:], ALU.add)
                nc.sync.dma_start(out=y_d[SUB*s:SUB*(s+1), :], in_=y_sb[:])

    nc.compile()
    return nc


_NC_CACHE = None


def kernel(**inputs):
    global _NC_CACHE
    from concourse.bass_utils import run_bass_kernel_spmd

    if _NC_CACHE is None:
        _NC_CACHE = build_bass()
    nc = _NC_CACHE

    x = np.ascontiguousarray(inputs["x"], np.float32)
    shared = {k: np.ascontiguousarray(np.asarray(inputs[k]), np.float32)
              for k in ("conv_w0", "conv_w1", "conv_w2", "conv_w3",
                        "fc_w", "fc_b")}
    in_maps = [
        {"x": x[i*B_PER_CORE:(i+1)*B_PER_CORE], **shared} for i in range(CORES)
    ]
    res = run_bass_kernel_spmd(nc, in_maps, core_ids=list(range(CORES)))
    return np.concatenate([r["y"] for r in res.results], axis=0)
